# revision 1
# baseline (speedup 1.0000x reference)
"""Chamfer loss (single-direction) Trainium2 Bass kernel.

Problem: pc_src [B=4, 3, M=8192], pc_dst [B=4, 3, N=8192] (fp32).
  d2[b,m,n] = ||src[b,:,m] - dst[b,:,n]||^2
  out = mean over (b,m) of sqrt(min_n d2[b,m,n])

Sharding: 8 cores = 4 batches x 2 M-halves. Each core handles one batch's
dst [3, 8192] and a 4096-point slice of that batch's src. The min over n is
complete per core; the host concatenates per-core min-d2 vectors and does
the (tiny, O(B*M)) sqrt + mean.

Device algorithm per core:
  Augmented K=9 matmul computes d2 exactly on the TensorEngine:
    lhsT rows (stationary, per 128-col src tile): [-2*s_x, -2*s_y, -2*s_z,
                                                   s_x^2, s_y^2, s_z^2, 1, 1, 1]
    rhs rows  (moving, dst):                      [d_x, d_y, d_z,
                                                   1, 1, 1, d_x^2, d_y^2, d_z^2]
    => psum[m, n] = -2*s.d + ||s||^2 + ||d||^2 = d2[m, n]
  The min-reduce runs on the VectorEngine with tensor_tensor_reduce, one
  instruction per pair of [128, 1024] PSUM tiles:
    accum = min(scalar_init, min_free(min(psumA, psumB)))
  which consumes 2 distance elements per cycle per lane (both read ports).
"""

import numpy as np

import concourse.bass as bass
import concourse.mybir as mybir
from concourse import bacc
from concourse import dve_ops as _dve_ops
from concourse.bass_utils import run_bass_kernel_spmd
from concourse.dve_spec import AluOp, C0, Spec, Src0, Src1, lower, minn
from concourse.dve_uop import DveOpSpec
from concourse.tile import TileContext

F32 = mybir.dt.float32
BIG = 3.0e38


def _make_min2_op():
    """Register a custom DVE op: out = min(in0, in1); accum_out = min(s0, min_k out).

    One DVE instruction consumes two fresh fp32 streams per cycle per lane
    (both read ports) AND folds the running minimum — the stock ISA
    tensor_tensor_reduce opcode has no ucode behind it on this target, and
    stock tensor_reduce is single-stream.
    """
    name = "MIN2_REDUCE_ANT"
    for existing in _dve_ops.OPS:
        if existing.name == name:
            return existing
    spec = Spec(
        body=minn(Src0, Src1),
        accum=AluOp.MIN,
        accum_init=C0,
        reference=lambda in0, in1, c0, c1, c2: (
            np.minimum(in0, in1),
            np.minimum(
                np.asarray(c0, np.float32).reshape(-1, 1)
                if isinstance(c0, np.ndarray)
                else np.float32(c0),
                np.minimum(in0, in1).min(axis=-1, keepdims=True),
            )
            * np.ones((in0.shape[0], 1), np.float32),
        ),
    )
    opcode = _dve_ops._CUSTOM_DVE_ROW_BASE + len(_dve_ops.OPS)
    shas = {}
    for ver in ("v3", "v4"):
        try:
            tmp = DveOpSpec(
                name=name,
                opcode=opcode,
                uops=lower(spec, ver=ver),
                rd1_en=_dve_ops.has_src1(spec),
            )
            shas[ver] = tmp.sha(ver)
        except Exception:
            pass
    op = _dve_ops.DveOp(name, spec, subdim=False, uops_sha=shas)
    _dve_ops.OPS.append(op)
    _dve_ops.CUSTOM_DVE_SPECS[name] = spec
    _dve_ops._SUB_OPCODE_FOR_NAME[name] = opcode
    return op


MIN2 = _make_min2_op()

# Problem constants (hardcoded per contract)
B = 4
D = 3
M = 8192
N = 8192
N_CORES = 8
M_SHARD = M // 2  # 4096 src points per core

P = 128          # output partitions per M-tile
MM_N = 512       # matmul moving free dim (fp32 max; 1 PSUM bank)
PSUM_FD = 1024   # TTR operand width (2 PSUM banks)


def build_nc(m_shard: int = M_SHARD, n: int = N, reps: int = 1) -> bass.Bass:
    """reps>1 repeats the main loop (identical work) — used only by the test
    harness to measure steady-state HW time via the wall-clock slope."""
    assert m_shard % P == 0 and n % (4 * PSUM_FD) == 0
    m_tiles = m_shard // P
    pairs = n // (2 * PSUM_FD)  # TTR pairs per M-tile

    # Bacc (not plain Bass): its compile() pass splits multi-sem waits into
    # EventSemaphore instructions — TRN2 instructions hold only one wait.
    nc = bacc.Bacc()
    src = nc.dram_tensor("src", [D, m_shard], F32, kind="ExternalInput")
    dst = nc.dram_tensor("dst", [D, n], F32, kind="ExternalInput")
    out = nc.dram_tensor("out", [P, m_tiles], F32, kind="ExternalOutput")
    ones3 = nc.inline_tensor(np.ones((D, n), np.float32), "ones3")

    # Compute-engine APs need 32-aligned start partitions, so the three
    # computed row-triples live at partition bases 0 / 32 / 64 and the rows
    # in between are zeroed (they then contribute 0*0 to the contraction;
    # matmul cost only depends on the moving free dim, not K).
    KA = 2 * 32 + D  # 67 contraction rows

    with TileContext(nc) as tc:
        with (
            tc.tile_pool(name="big", bufs=1) as big,
            tc.tile_pool(name="scr", bufs=3) as scr,
            tc.tile_pool(name="psum", bufs=4, space="PSUM") as psum,
        ):
            # src rows: [-2s (0-2) | zeros | s^2 (32-34) | zeros | ones (64-66)]
            # dst rows: [d (0-2)   | zeros | ones (32-34)| zeros | d^2 (64-66)]
            srcT = big.tile([KA, m_shard], F32)
            dstT = big.tile([KA, n], F32)
            mins = big.tile([P, m_tiles], F32)

            # --- dstT ------------------------------------------------------
            n_chunk = 2048
            for c0 in range(0, n, n_chunk):
                cs = slice(c0, c0 + n_chunk)
                nc.gpsimd.memset(dstT[0:32, cs], 0.0)
                nc.vector.memset(dstT[32:64, cs], 0.0)
                nc.sync.dma_start(out=dstT[0:D, cs], in_=dst[:, cs])
                nc.sync.dma_start(out=dstT[32 : 32 + D, cs], in_=ones3[:, cs])
                nc.sync.dma_start(out=dstT[64 : 64 + D, cs], in_=dst[:, cs])
                nc.scalar.activation(
                    out=dstT[64 : 64 + D, cs],
                    in_=dstT[64 : 64 + D, cs],
                    func=mybir.ActivationFunctionType.Square,
                )

            # --- srcT ------------------------------------------------------
            m_chunk = min(2048, m_shard)
            for c0 in range(0, m_shard, m_chunk):
                cs = slice(c0, c0 + m_chunk)
                nc.gpsimd.memset(srcT[0:32, cs], 0.0)
                nc.gpsimd.memset(srcT[32:64, cs], 0.0)
                nc.sync.dma_start(out=srcT[0:D, cs], in_=src[:, cs])
                nc.sync.dma_start(out=srcT[32 : 32 + D, cs], in_=src[:, cs])
                nc.sync.dma_start(out=srcT[64 : 64 + D, cs], in_=ones3[:, : m_chunk])
                nc.vector.tensor_scalar_mul(srcT[0:D, cs], srcT[0:D, cs], -2.0)
                nc.scalar.activation(
                    out=srcT[32 : 32 + D, cs],
                    in_=srcT[32 : 32 + D, cs],
                    func=mybir.ActivationFunctionType.Square,
                )

            # --- main loop: 1 M-tile = 128 src points vs all n dst points -
            for mt in [t for _ in range(reps) for t in range(m_tiles)]:
                lhsT = srcT[:, mt * P : (mt + 1) * P]  # [9, 128]
                for pr in range(pairs):
                    base = pr * 2 * PSUM_FD
                    pA = psum.tile([P, PSUM_FD], F32, tag="ps")
                    pB = psum.tile([P, PSUM_FD], F32, tag="ps")
                    for t, pt in ((0, pA), (1, pB)):
                        for h in range(PSUM_FD // MM_N):
                            n0 = base + t * PSUM_FD + h * MM_N
                            nc.tensor.matmul(
                                pt[:, h * MM_N : (h + 1) * MM_N],
                                lhsT,
                                dstT[:, n0 : n0 + MM_N],
                                start=True,
                                stop=True,
                            )
                    # ISA: only one non-scalar input may live in PSUM, so the
                    # (otherwise idle) ScalarE stages pB into SBUF first.
                    sB = scr.tile([P, PSUM_FD], F32, tag="cp")
                    nc.scalar.copy(out=sB, in_=pB)
                    ttr_out = scr.tile([P, PSUM_FD], F32, tag="ttr")
                    init = BIG if pr == 0 else mins[:, mt : mt + 1]
                    nc.vector._custom_dve(
                        MIN2,
                        out=ttr_out,
                        in0=pA,
                        in1=sB,
                        s0=init,
                        accum_out=mins[:, mt : mt + 1],
                    )

            nc.sync.dma_start(out=out[:, :], in_=mins[:, :])

    nc.finalize()
    return nc


_NC_CACHE: dict = {}


def _get_nc(m_shard: int, n: int) -> bass.Bass:
    key = (m_shard, n)
    if key not in _NC_CACHE:
        _NC_CACHE[key] = build_nc(m_shard, n)
    return _NC_CACHE[key]


LAST_RESULTS = None  # test harness can inspect exec_time_ns etc.


def kernel(pc_src: np.ndarray, pc_dst: np.ndarray) -> np.ndarray:
    pc_src = np.ascontiguousarray(np.asarray(pc_src), dtype=np.float32)
    pc_dst = np.ascontiguousarray(np.asarray(pc_dst), dtype=np.float32)
    assert pc_src.shape == (B, D, M) and pc_dst.shape == (B, D, N)

    nc = _get_nc(M_SHARD, N)

    in_maps = []
    for c in range(N_CORES):
        b, h = divmod(c, 2)
        in_maps.append(
            {
                "src": np.ascontiguousarray(pc_src[b, :, h * M_SHARD : (h + 1) * M_SHARD]),
                "dst": np.ascontiguousarray(pc_dst[b]),
            }
        )

    global LAST_RESULTS
    LAST_RESULTS = run_bass_kernel_spmd(nc, in_maps, core_ids=list(range(N_CORES)))

    # host: O(B*M) postprocess (sqrt + mean) over per-core min-d2 columns
    md2 = np.concatenate(
        [LAST_RESULTS.results[c]["out"].T.reshape(-1) for c in range(N_CORES)]
    )
    md2 = np.maximum(md2, 0.0)
    dists = np.sqrt(md2, dtype=np.float32)
    return np.asarray(np.mean(dists, dtype=np.float32), dtype=np.float32)



# revision 3
# speedup vs baseline: 2.8148x; 2.8148x over previous
"""Chamfer loss (single-direction) Trainium2 Bass kernel.

Problem: pc_src [B=4, 3, M=8192], pc_dst [B=4, 3, N=8192] (fp32).
  d2[b,m,n] = ||src[b,:,m] - dst[b,:,n]||^2
  out = mean over (b,m) of sqrt(min_n d2[b,m,n])

Sharding: 8 cores = 4 batches x 2 M-halves. Each core handles one batch's
dst [3, 8192] and a 4096-point slice of that batch's src. The min over n is
complete per core; the host concatenates per-core min-d2 vectors and does
the (tiny, O(B*M)) sqrt + mean.

Device algorithm per core:
  Host augments both point clouds to K=5 contraction rows so a single
  fp32r matmul computes d2 exactly on the TensorEngine:
    lhsT rows (stationary, per 128-col src tile): [-2*s_x, -2*s_y, -2*s_z,
                                                   ||s||^2, 1]
    rhs rows  (moving, dst):                      [d_x, d_y, d_z,
                                                   1, ||d||^2]
    => psum[m, n] = -2*s.d + ||s||^2 + ||d||^2 = d2[m, n]
  fp32r runs the PE at 1 cycle/row (vs 4 for fp32) when the moving free
  dim is >= 256. The min-reduce runs on the VectorEngine with a custom
  dual-stream DVE op, one instruction per pair of [128, 1024] tiles:
    accum = min(scalar_init, min_free(min(psumA, sbufB)))
  consuming 2 distance elements per cycle per lane; the otherwise-idle
  ScalarEngine stages the second stream PSUM->SBUF.
"""

import numpy as np

import concourse.bass as bass
import concourse.mybir as mybir
from concourse import bacc
from concourse import dve_ops as _dve_ops
from concourse.bass_utils import run_bass_kernel_spmd
from concourse.dve_spec import AluOp, C0, Spec, Src0, Src1, lower, minn
from concourse.dve_uop import DveOpSpec
from concourse.tile import TileContext

F32 = mybir.dt.float32
F32R = mybir.dt.float32r
BIG = 3.0e38


def _make_min2_op():
    """Register a custom DVE op: out = min(in0, in1); accum_out = min(s0, min_k out).

    One DVE instruction consumes two fresh fp32 streams per cycle per lane
    (both read ports) AND folds the running minimum — the stock ISA
    tensor_tensor_reduce opcode has no ucode behind it on this target, and
    stock tensor_reduce is single-stream.
    """
    name = "MIN2_REDUCE_ANT"
    for existing in _dve_ops.OPS:
        if existing.name == name:
            return existing
    spec = Spec(
        body=minn(Src0, Src1),
        accum=AluOp.MIN,
        accum_init=C0,
        reference=lambda in0, in1, c0, c1, c2: (
            np.minimum(in0, in1),
            np.minimum(
                np.asarray(c0, np.float32).reshape(-1, 1)
                if isinstance(c0, np.ndarray)
                else np.float32(c0),
                np.minimum(in0, in1).min(axis=-1, keepdims=True),
            )
            * np.ones((in0.shape[0], 1), np.float32),
        ),
    )
    opcode = _dve_ops._CUSTOM_DVE_ROW_BASE + len(_dve_ops.OPS)
    shas = {}
    for ver in ("v3", "v4"):
        try:
            tmp = DveOpSpec(
                name=name,
                opcode=opcode,
                uops=lower(spec, ver=ver),
                rd1_en=_dve_ops.has_src1(spec),
            )
            shas[ver] = tmp.sha(ver)
        except Exception:
            pass
    op = _dve_ops.DveOp(name, spec, subdim=False, uops_sha=shas)
    _dve_ops.OPS.append(op)
    _dve_ops.CUSTOM_DVE_SPECS[name] = spec
    _dve_ops._SUB_OPCODE_FOR_NAME[name] = opcode
    return op


MIN2 = _make_min2_op()

# Problem constants (hardcoded per contract)
B = 4
D = 3
M = 8192
N = 8192
N_CORES = 8
M_SHARD = M // 2  # 4096 src points per core
K = 5            # contraction rows of the augmented matmul

P = 128          # output partitions per M-tile
MM_N = 512       # matmul moving free dim (fp32 max; 1 PSUM bank)
PSUM_FD = 1024   # dual-stream min operand width (2 PSUM banks)

# True: DMA lands in fp32 tiles and a per-chunk compute copy rounds into the
# f32r matmul operands (walrus wants a rounding producer). False: DMA writes
# the f32r tiles directly.
ROUND_ON_DEVICE = True


def build_nc(m_shard: int = M_SHARD, n: int = N, reps: int = 1) -> bass.Bass:
    """reps>1 repeats the main loop (identical work) — used only by the test
    harness to measure steady-state HW time via the wall-clock slope."""
    assert m_shard % P == 0 and n % (4 * PSUM_FD) == 0
    m_tiles = m_shard // P
    pairs = n // (2 * PSUM_FD)  # dual-stream min pairs per M-tile

    # Bacc (not plain Bass): its compile() pass splits multi-sem waits into
    # EventSemaphore instructions — TRN2 instructions hold only one wait.
    nc = bacc.Bacc()
    src = nc.dram_tensor("src", [K, m_shard], F32, kind="ExternalInput")
    dst = nc.dram_tensor("dst", [K, n], F32, kind="ExternalInput")
    out = nc.dram_tensor("out", [P, m_tiles], F32, kind="ExternalOutput")

    with TileContext(nc) as tc:
        with (
            tc.tile_pool(name="big", bufs=1) as big,
            tc.tile_pool(name="scr", bufs=3) as scr,
            tc.tile_pool(name="psum", bufs=4, space="PSUM") as psum,
        ):
            srcT = big.tile([K, m_shard], F32R)
            dstT = big.tile([K, n], F32R)
            mins = big.tile([P, m_tiles], F32)

            if ROUND_ON_DEVICE:
                srcS = big.tile([K, m_shard], F32)
                dstS = big.tile([K, n], F32)
                nc.sync.dma_start(out=srcS, in_=src[:, :])
                nc.sync.dma_start(out=dstS, in_=dst[:, :])
                # Chunked rounding copies so the first matmuls can start
                # before the whole staging pass finishes; split across the
                # two otherwise-idle-at-this-point engines.
                cw = 2048
                for c0 in range(0, n, cw):
                    cs = slice(c0, c0 + cw)
                    nc.scalar.copy(out=dstT[:, cs], in_=dstS[:, cs])
                for c0 in range(0, m_shard, cw):
                    cs = slice(c0, c0 + cw)
                    nc.vector.tensor_copy(out=srcT[:, cs], in_=srcS[:, cs])
            else:
                nc.sync.dma_start(out=srcT, in_=src[:, :].bitcast(F32R))
                nc.sync.dma_start(out=dstT, in_=dst[:, :].bitcast(F32R))

            # --- main loop: 1 M-tile = 128 src points vs all n dst points -
            for mt in [t for _ in range(reps) for t in range(m_tiles)]:
                lhsT = srcT[:, mt * P : (mt + 1) * P]  # [5, 128]
                for pr in range(pairs):
                    base = pr * 2 * PSUM_FD
                    pA = psum.tile([P, PSUM_FD], F32, tag="ps")
                    pB = psum.tile([P, PSUM_FD], F32, tag="ps")
                    for t, pt in ((0, pA), (1, pB)):
                        for h in range(PSUM_FD // MM_N):
                            n0 = base + t * PSUM_FD + h * MM_N
                            nc.tensor.matmul(
                                pt[:, h * MM_N : (h + 1) * MM_N],
                                lhsT,
                                dstT[:, n0 : n0 + MM_N],
                                start=True,
                                stop=True,
                            )
                    # ISA: only one non-scalar input may live in PSUM, so the
                    # (otherwise idle) ScalarE stages pB into SBUF first.
                    sB = scr.tile([P, PSUM_FD], F32, tag="cp")
                    nc.scalar.copy(out=sB, in_=pB)
                    ttr_out = scr.tile([P, PSUM_FD], F32, tag="ttr")
                    init = BIG if pr == 0 else mins[:, mt : mt + 1]
                    nc.vector._custom_dve(
                        MIN2,
                        out=ttr_out,
                        in0=pA,
                        in1=sB,
                        s0=init,
                        accum_out=mins[:, mt : mt + 1],
                    )

            nc.sync.dma_start(out=out[:, :], in_=mins[:, :])

    nc.finalize()
    return nc


_NC_CACHE: dict = {}


def _get_nc(m_shard: int, n: int) -> bass.Bass:
    key = (m_shard, n)
    if key not in _NC_CACHE:
        _NC_CACHE[key] = build_nc(m_shard, n)
    return _NC_CACHE[key]


def _augment(pc: np.ndarray, role: str) -> np.ndarray:
    """[3, n] fp32 -> [5, n] augmented rows (host, O(n))."""
    n = pc.shape[1]
    out = np.empty((K, n), np.float32)
    sq = np.sum(pc.astype(np.float64) ** 2, axis=0).astype(np.float32)
    if role == "src":
        out[0:3] = -2.0 * pc
        out[3] = sq
        out[4] = 1.0
    else:
        out[0:3] = pc
        out[3] = 1.0
        out[4] = sq
    return out


LAST_RESULTS = None  # test harness can inspect exec_time_ns etc.


def kernel(pc_src: np.ndarray, pc_dst: np.ndarray) -> np.ndarray:
    pc_src = np.ascontiguousarray(np.asarray(pc_src), dtype=np.float32)
    pc_dst = np.ascontiguousarray(np.asarray(pc_dst), dtype=np.float32)
    assert pc_src.shape == (B, D, M) and pc_dst.shape == (B, D, N)

    nc = _get_nc(M_SHARD, N)

    in_maps = []
    for c in range(N_CORES):
        b, h = divmod(c, 2)
        in_maps.append(
            {
                "src": _augment(pc_src[b, :, h * M_SHARD : (h + 1) * M_SHARD], "src"),
                "dst": _augment(pc_dst[b], "dst"),
            }
        )

    global LAST_RESULTS
    LAST_RESULTS = run_bass_kernel_spmd(nc, in_maps, core_ids=list(range(N_CORES)))

    # host: O(B*M) postprocess (sqrt + mean) over per-core min-d2 columns
    md2 = np.concatenate(
        [LAST_RESULTS.results[c]["out"].T.reshape(-1) for c in range(N_CORES)]
    )
    md2 = np.maximum(md2, 0.0)
    dists = np.sqrt(md2, dtype=np.float32)
    return np.asarray(np.mean(dists, dtype=np.float32), dtype=np.float32)


# revision 9
# speedup vs baseline: 3.6596x; 1.3001x over previous
"""Chamfer loss (single-direction) Trainium2 Bass kernel.

Problem: pc_src [B=4, 3, M=8192], pc_dst [B=4, 3, N=8192] (fp32).
  d2[b,m,n] = ||src[b,:,m] - dst[b,:,n]||^2
  out = mean over (b,m) of sqrt(min_n d2[b,m,n])

Sharding: 8 cores = 4 batches x 2 M-halves. Each core handles one batch's
dst [3, 8192] and a 4096-point slice of that batch's src. The min over n is
complete per core; the host concatenates per-core min-d2 vectors and does
the (tiny, O(B*M)) sqrt + mean.

Device algorithm per core:
  Host splits every augmented row into a bf16 hi+lo pair and lays out
  K=16 contraction rows so ONE bf16 matmul computes d2 to ~2^-17:
  with a = -2*s = a1+a2, d = d1+d2, ||s||^2 = ns1+ns2, ||d||^2 = nd1+nd2
  (each split exact in bf16 hi + bf16 residual):
    lhsT rows: [a1(3), a1(3), a2(3), a2(3), ns1, ns2, 1, 1]
    rhs  rows: [d1(3), d2(3), d1(3), d2(3), 1, 1, nd1, nd2]
    => psum[m, n] = (a1+a2).(d1+d2) + ||s||^2 + ||d||^2 = d2[m, n]
  bf16 runs the PE at 1 cycle/row with a 1024-wide moving operand (2x the
  fp32 limit) and fast weight loads. The min-reduce runs on the
  VectorEngine with a custom dual-stream DVE op, one instruction per pair
  of [128, 1024] tiles:
    accum = min(scalar_init, min_free(min(psumA, sbufB)))
  consuming 2 distance elements per cycle per lane; the otherwise-idle
  ScalarEngine stages the second stream PSUM->SBUF.
"""

import numpy as np

import concourse.bass as bass
import concourse.mybir as mybir
from concourse import bacc
from concourse import dve_ops as _dve_ops
from concourse.bass_utils import run_bass_kernel_spmd
from concourse.dve_spec import AluOp, C0, Spec, Src0, Src1, lower, minn
from concourse.dve_uop import DveOpSpec
from concourse.tile import TileContext

F32 = mybir.dt.float32
BF16 = mybir.dt.bfloat16
BIG = 3.0e38


def _make_min2_op():
    """Register a custom DVE op: out = min(in0, in1); accum_out = min(s0, min_k out).

    One DVE instruction consumes two fresh fp32 streams per cycle per lane
    (both read ports) AND folds the running minimum — the stock ISA
    tensor_tensor_reduce opcode has no ucode behind it on this target, and
    stock tensor_reduce is single-stream.
    """
    name = "MIN2_REDUCE_ANT"
    for existing in _dve_ops.OPS:
        if existing.name == name:
            return existing
    spec = Spec(
        body=minn(Src0, Src1),
        accum=AluOp.MIN,
        accum_init=C0,
        reference=lambda in0, in1, c0, c1, c2: (
            np.minimum(in0, in1),
            np.minimum(
                np.asarray(c0, np.float32).reshape(-1, 1)
                if isinstance(c0, np.ndarray)
                else np.float32(c0),
                np.minimum(in0, in1).min(axis=-1, keepdims=True),
            )
            * np.ones((in0.shape[0], 1), np.float32),
        ),
    )
    opcode = _dve_ops._CUSTOM_DVE_ROW_BASE + len(_dve_ops.OPS)
    shas = {}
    for ver in ("v3", "v4"):
        try:
            tmp = DveOpSpec(
                name=name,
                opcode=opcode,
                uops=lower(spec, ver=ver),
                rd1_en=_dve_ops.has_src1(spec),
            )
            shas[ver] = tmp.sha(ver)
        except Exception:
            pass
    op = _dve_ops.DveOp(name, spec, subdim=False, uops_sha=shas)
    _dve_ops.OPS.append(op)
    _dve_ops.CUSTOM_DVE_SPECS[name] = spec
    _dve_ops._SUB_OPCODE_FOR_NAME[name] = opcode
    return op


MIN2 = _make_min2_op()

# Problem constants (hardcoded per contract)
B = 4
D = 3
M = 8192
N = 8192
N_CORES = 8
M_SHARD = M // 2  # 4096 src points per core
K = 16           # contraction rows of the hi/lo-split augmented matmul

P = 128          # output partitions per M-tile
MM_N = 512       # matmul moving free dim (PSUM-bank limit: 512 fp32 out)
PSUM_FD = 1024   # dual-stream min operand width (2 PSUM banks)


def build_nc(m_shard: int = M_SHARD, n: int = N, reps: int = 1) -> bass.Bass:
    """reps>1 repeats the main loop (identical work) — used only by the test
    harness to measure steady-state HW time via the wall-clock slope."""
    assert m_shard % P == 0 and n % (4 * PSUM_FD) == 0
    m_tiles = m_shard // P
    pairs = n // (2 * PSUM_FD)  # dual-stream min pairs per M-tile

    # Bacc (not plain Bass): its compile() pass splits multi-sem waits into
    # EventSemaphore instructions — TRN2 instructions hold only one wait.
    nc = bacc.Bacc()
    src = nc.dram_tensor("src", [K, m_shard], BF16, kind="ExternalInput")
    dst = nc.dram_tensor("dst", [K, n], BF16, kind="ExternalInput")
    out = nc.dram_tensor("out", [P, m_tiles], F32, kind="ExternalOutput")

    with TileContext(nc) as tc:
        with (
            tc.tile_pool(name="big", bufs=1) as big,
            tc.tile_pool(name="scr", bufs=3) as scr,
            tc.tile_pool(name="psum", bufs=4, space="PSUM") as psum,
        ):
            srcT = big.tile([K, m_shard], BF16)
            dstT = big.tile([K, n], BF16)
            mins = big.tile([P, m_tiles], F32)

            nc.sync.dma_start(out=srcT, in_=src[:, :])
            nc.sync.dma_start(out=dstT, in_=dst[:, :])

            # --- main loop: 1 M-tile = 128 src points vs all n dst points -
            for mt in [t for _ in range(reps) for t in range(m_tiles)]:
                lhsT = srcT[:, mt * P : (mt + 1) * P]  # [16, 128]
                for pr in range(pairs):
                    base = pr * 2 * PSUM_FD
                    pA = psum.tile([P, PSUM_FD], F32, tag="ps")
                    pB = psum.tile([P, PSUM_FD], F32, tag="ps")
                    for t, pt in ((0, pA), (1, pB)):
                        for h in range(PSUM_FD // MM_N):
                            n0 = base + t * PSUM_FD + h * MM_N
                            nc.tensor.matmul(
                                pt[:, h * MM_N : (h + 1) * MM_N],
                                lhsT,
                                dstT[:, n0 : n0 + MM_N],
                                start=True,
                                stop=True,
                            )
                    # ISA: only one non-scalar input may live in PSUM, so the
                    # (otherwise idle) ScalarE stages pB into SBUF first.
                    sB = scr.tile([P, PSUM_FD], F32, tag="cp")
                    nc.scalar.copy(out=sB, in_=pB)
                    ttr_out = scr.tile([P, PSUM_FD], F32, tag="ttr")
                    init = BIG if pr == 0 else mins[:, mt : mt + 1]
                    nc.vector._custom_dve(
                        MIN2,
                        out=ttr_out,
                        in0=pA,
                        in1=sB,
                        s0=init,
                        accum_out=mins[:, mt : mt + 1],
                    )

            nc.sync.dma_start(out=out[:, :], in_=mins[:, :])

    nc.finalize()
    return nc


_NC_CACHE: dict = {}


def _get_nc(m_shard: int, n: int) -> bass.Bass:
    key = (m_shard, n)
    if key not in _NC_CACHE:
        _NC_CACHE[key] = build_nc(m_shard, n)
    return _NC_CACHE[key]


def _split(x: np.ndarray) -> tuple[np.ndarray, np.ndarray]:
    """fp32 -> (bf16 hi, bf16 lo) with x ~= hi + lo to ~2^-17 rel."""
    import ml_dtypes

    hi = x.astype(ml_dtypes.bfloat16)
    lo = (x - hi.astype(np.float32)).astype(ml_dtypes.bfloat16)
    return hi, lo


def _augment(pc: np.ndarray, role: str) -> np.ndarray:
    """[3, n] fp32 -> [16, n] bf16 hi/lo-split augmented rows (host, O(n))."""
    import ml_dtypes

    n = pc.shape[1]
    out = np.empty((K, n), ml_dtypes.bfloat16)
    sq = np.sum(pc.astype(np.float64) ** 2, axis=0).astype(np.float32)
    sq1, sq2 = _split(sq)
    if role == "src":
        a1, a2 = _split(-2.0 * pc)
        out[0:3] = a1
        out[3:6] = a1
        out[6:9] = a2
        out[9:12] = a2
        out[12] = sq1
        out[13] = sq2
        out[14] = 1.0
        out[15] = 1.0
    else:
        d1, d2 = _split(pc)
        out[0:3] = d1
        out[3:6] = d2
        out[6:9] = d1
        out[9:12] = d2
        out[12] = 1.0
        out[13] = 1.0
        out[14] = sq1
        out[15] = sq2
    return out


LAST_RESULTS = None  # test harness can inspect exec_time_ns etc.


def kernel(pc_src: np.ndarray, pc_dst: np.ndarray) -> np.ndarray:
    pc_src = np.ascontiguousarray(np.asarray(pc_src), dtype=np.float32)
    pc_dst = np.ascontiguousarray(np.asarray(pc_dst), dtype=np.float32)
    assert pc_src.shape == (B, D, M) and pc_dst.shape == (B, D, N)

    nc = _get_nc(M_SHARD, N)

    in_maps = []
    for c in range(N_CORES):
        b, h = divmod(c, 2)
        in_maps.append(
            {
                "src": _augment(pc_src[b, :, h * M_SHARD : (h + 1) * M_SHARD], "src"),
                "dst": _augment(pc_dst[b], "dst"),
            }
        )

    global LAST_RESULTS
    LAST_RESULTS = run_bass_kernel_spmd(nc, in_maps, core_ids=list(range(N_CORES)))

    # host: O(B*M) postprocess (sqrt + mean) over per-core min-d2 columns
    md2 = np.concatenate(
        [LAST_RESULTS.results[c]["out"].T.reshape(-1) for c in range(N_CORES)]
    )
    md2 = np.maximum(md2, 0.0)
    dists = np.sqrt(md2, dtype=np.float32)
    return np.asarray(np.mean(dists, dtype=np.float32), dtype=np.float32)
